# revision 5
# baseline (speedup 1.0000x reference)
"""Trainium2 Bass kernel for nn_CrossAttentionGating.

Sharding: data-parallel over batch B=8 across 8 cores (1 batch element per
core); all weights replicated. Host numpy does all layout prep (transposes,
chunking into 128-partition tiles, additive attention mask from lengths).

Per-core device pipeline (D=512 split into 4 chunks of 128 partitions):
  qp_T[d,q] = Wq^T.T @ audio^T          (PE, f32r)
  kp_T[d,k] = Wkv^T.T @ text^T + b_attn (PE + ACT bias)
  for each k:  X = qp_T + kp_T[:,k]     (DVE tensor_scalar per-partition add)
               H = tanh(X)              (ACT, batched big instructions)
               score_T[k,:] += v_c.T@H  (PE, M=1 f32r matmuls, PSUM accum)
  softmax over k: PE-transpose score to [q,k], +mask, max/exp/sum/recip
  ctx_T[e,q] = text.T @ attn_T          (PE)
  g_u = sigmoid(Wu^T.T @ audio^T + b_u); s_out_T = ctx_T * g_u
  g_s = sigmoid(Ws^T.T @ ctx_T + b_s);   u_out_T = audio_T * g_s
"""

import sys

for _p in ("/opt/trn_rl_repo", "/opt/pypackages"):
    if _p not in sys.path:
        sys.path.append(_p)

from contextlib import ExitStack

import numpy as np

import concourse.bacc as bacc
import concourse.tile as tile
import concourse.mybir as mybir
from concourse import masks
from concourse.bass_utils import run_bass_kernel_spmd

B, TQ, TK, D = 8, 512, 64, 512
P = 128
NC = D // P  # 4 chunks of the embedding dim
KB = 2      # k's per tanh batch
NEG = -1e10
F32 = mybir.dt.float32
F32R = mybir.dt.float32r
AF = mybir.ActivationFunctionType

TRACE = False
LAST_EXEC_NS = None

_cached_nc = None


def _build():
    nc = bacc.Bacc("TRN2", target_bir_lowering=False, debug=False, num_devices=B)

    audio3 = nc.dram_tensor("audio3", [P, NC, TQ], F32R, kind="ExternalInput")
    wq3 = nc.dram_tensor("wq3", [P, NC, D], F32R, kind="ExternalInput")
    wkv3 = nc.dram_tensor("wkv3", [P, NC, D], F32R, kind="ExternalInput")
    wu3 = nc.dram_tensor("wu3", [P, NC, D], F32R, kind="ExternalInput")
    ws3 = nc.dram_tensor("ws3", [P, NC, D], F32R, kind="ExternalInput")
    text2 = nc.dram_tensor("text2", [TK, D], F32R, kind="ExternalInput")
    text3 = nc.dram_tensor("text3", [P, NC, TK], F32R, kind="ExternalInput")
    battn_c = nc.dram_tensor("battn_c", [P, NC], F32, kind="ExternalInput")
    bu_c = nc.dram_tensor("bu_c", [P, NC], F32, kind="ExternalInput")
    bs_c = nc.dram_tensor("bs_c", [P, NC], F32, kind="ExternalInput")
    v_c = nc.dram_tensor("v_c", [P, NC, 2], F32R, kind="ExternalInput")
    mask3 = nc.dram_tensor("mask3", [P, NC, TK], F32, kind="ExternalInput")
    uoutT = nc.dram_tensor("uoutT", [P, NC, TQ], F32, kind="ExternalOutput")
    soutT = nc.dram_tensor("soutT", [P, NC, TQ], F32, kind="ExternalOutput")

    with tile.TileContext(nc) as tc, ExitStack() as ctx:
        cpool = ctx.enter_context(tc.tile_pool(name="const", bufs=1))
        ppool = ctx.enter_context(tc.tile_pool(name="ps", bufs=4, space="PSUM"))
        spool = ctx.enter_context(tc.tile_pool(name="score", bufs=1, space="PSUM"))
        xpool = ctx.enter_context(tc.tile_pool(name="x", bufs=2))
        hpool = ctx.enter_context(tc.tile_pool(name="h", bufs=2))
        wpool = ctx.enter_context(tc.tile_pool(name="work", bufs=2))

        # ---- persistent loads ----
        audio_sb = cpool.tile([P, NC, TQ], F32R)
        nc.sync.dma_start(audio_sb[:], audio3[:])
        wq_sb = cpool.tile([P, NC, D], F32R)
        nc.sync.dma_start(wq_sb[:], wq3[:])
        wkv_sb = cpool.tile([P, NC, D], F32R)
        nc.sync.dma_start(wkv_sb[:], wkv3[:])
        wu_sb = cpool.tile([P, NC, D], F32R)
        nc.sync.dma_start(wu_sb[:], wu3[:])
        ws_sb = cpool.tile([P, NC, D], F32R)
        nc.sync.dma_start(ws_sb[:], ws3[:])
        text_sb = cpool.tile([TK, D], F32R)
        nc.sync.dma_start(text_sb[:], text2[:])
        text3_sb = cpool.tile([P, NC, TK], F32R)
        nc.sync.dma_start(text3_sb[:], text3[:])
        battn_sb = cpool.tile([P, NC], F32)
        nc.sync.dma_start(battn_sb[:], battn_c[:])
        bu_sb = cpool.tile([P, NC], F32)
        nc.sync.dma_start(bu_sb[:], bu_c[:])
        bs_sb = cpool.tile([P, NC], F32)
        nc.sync.dma_start(bs_sb[:], bs_c[:])
        v_sb = cpool.tile([P, NC, 2], F32R)
        nc.sync.dma_start(v_sb[:], v_c[:])
        mask_sb = cpool.tile([P, NC, TK], F32)
        nc.sync.dma_start(mask_sb[:], mask3[:])

        ident = cpool.tile([P, P], F32)
        masks.make_identity(nc, ident[:])

        # ---- projections ----
        kp_sb = cpool.tile([P, NC, TK], F32)
        for dc in range(NC):
            kp_ps = ppool.tile([P, TK], F32, tag="ps")
            for ec in range(NC):
                nc.tensor.matmul(
                    kp_ps[:],
                    wkv_sb[:, ec, dc * P:(dc + 1) * P],
                    text3_sb[:, ec, :],
                    start=(ec == 0),
                    stop=(ec == NC - 1),
                )
            nc.scalar.activation(
                kp_sb[:, dc, :], kp_ps[:], AF.Identity, bias=battn_sb[:, dc:dc + 1]
            )

        qp_sb = cpool.tile([P, NC, TQ], F32)
        for dc in range(NC):
            qp_ps = ppool.tile([P, TQ], F32, tag="ps")
            for ec in range(NC):
                nc.tensor.matmul(
                    qp_ps[:],
                    wq_sb[:, ec, dc * P:(dc + 1) * P],
                    audio_sb[:, ec, :],
                    start=(ec == 0),
                    stop=(ec == NC - 1),
                )
            nc.vector.tensor_copy(qp_sb[:, dc, :], qp_ps[:])

        # ---- scores: score[q, k] = v . tanh(qp_T[:,q] + kp_T[:,k]) ----
        # One PSUM bank holds the whole [q-chunk, qc, k] score block; each
        # matmul is lhsT=H[128d,128q] x rhs=v[128,1] -> [128q,1] accumulated
        # over the 4 d-chunks.
        score_ps = spool.tile([P, NC, TK, 2], F32)
        for kb in range(TK // KB):
            x_t = xpool.tile([P, KB, NC, TQ], F32, tag="x")
            for kk in range(KB):
                k = kb * KB + kk
                for dc in range(NC):
                    nc.vector.tensor_scalar_add(
                        x_t[:, kk, dc, :], qp_sb[:, dc, :], kp_sb[:, dc, k:k + 1]
                    )
            h_t = hpool.tile([P, KB, NC, TQ], F32R, tag="h")
            nc.scalar.activation(h_t[:], x_t[:], AF.Tanh)
            for kk in range(KB):
                k = kb * KB + kk
                for qc in range(NC):
                    for dc in range(NC):
                        nc.tensor.matmul(
                            score_ps[:, qc, k, :],
                            h_t[:, kk, dc, qc * P:(qc + 1) * P],
                            v_sb[:, dc, :],
                            start=(dc == 0),
                            stop=(dc == NC - 1),
                        )

        # ---- softmax over k (already in [q, k] layout) ----
        sm_sb = cpool.tile([P, NC, TK], F32)
        e_sb = cpool.tile([P, NC, TK], F32)
        attn_sb = cpool.tile([P, NC, TK], F32)
        attnT_sb = cpool.tile([TK, TQ], F32R)
        for qc in range(NC):
            nc.vector.tensor_add(sm_sb[:, qc, :], score_ps[:, qc, :, 0], mask_sb[:, qc, :])
            nmax = wpool.tile([P, 1], F32, tag="nmax")
            nc.vector.reduce_max(
                nmax[:], sm_sb[:, qc, :], axis=mybir.AxisListType.X, negate=True
            )
            nc.scalar.activation(e_sb[:, qc, :], sm_sb[:, qc, :], AF.Exp, bias=nmax[:])
            ssum = wpool.tile([P, 1], F32, tag="ssum")
            nc.vector.reduce_sum(ssum[:], e_sb[:, qc, :], axis=mybir.AxisListType.X)
            rinv = wpool.tile([P, 1], F32, tag="rinv")
            nc.vector.reciprocal(rinv[:], ssum[:])
            nc.vector.tensor_scalar_mul(attn_sb[:, qc, :], e_sb[:, qc, :], rinv[:])
            at_ps = ppool.tile([TK, P], F32, tag="ps")
            nc.tensor.transpose(at_ps[:], attn_sb[:, qc, :], ident[:])
            nc.vector.tensor_copy(attnT_sb[:, qc * P:(qc + 1) * P], at_ps[:])

        # ---- ctx_T[e, q] = text.T @ attn_T ----
        ctx_sb = cpool.tile([P, NC, TQ], F32R)
        for ec in range(NC):
            ctx_ps = ppool.tile([P, TQ], F32, tag="ps")
            nc.tensor.matmul(
                ctx_ps[:],
                text_sb[:, ec * P:(ec + 1) * P],
                attnT_sb[:],
                start=True,
                stop=True,
            )
            nc.vector.tensor_copy(ctx_sb[:, ec, :], ctx_ps[:])

        # ---- gating ----
        for dc in range(NC):
            gu_ps = ppool.tile([P, TQ], F32, tag="ps")
            for ec in range(NC):
                nc.tensor.matmul(
                    gu_ps[:],
                    wu_sb[:, ec, dc * P:(dc + 1) * P],
                    audio_sb[:, ec, :],
                    start=(ec == 0),
                    stop=(ec == NC - 1),
                )
            gu_sb = wpool.tile([P, TQ], F32, tag="gu")
            nc.scalar.activation(gu_sb[:], gu_ps[:], AF.Sigmoid, bias=bu_sb[:, dc:dc + 1])
            so_sb = wpool.tile([P, TQ], F32, tag="so")
            nc.vector.tensor_mul(so_sb[:], ctx_sb[:, dc, :].bitcast(F32), gu_sb[:])
            nc.sync.dma_start(soutT[:, dc, :], so_sb[:])

        for dc in range(NC):
            gs_ps = ppool.tile([P, TQ], F32, tag="ps")
            for ec in range(NC):
                nc.tensor.matmul(
                    gs_ps[:],
                    ws_sb[:, ec, dc * P:(dc + 1) * P],
                    ctx_sb[:, ec, :],
                    start=(ec == 0),
                    stop=(ec == NC - 1),
                )
            gs_sb = wpool.tile([P, TQ], F32, tag="gs")
            nc.scalar.activation(gs_sb[:], gs_ps[:], AF.Sigmoid, bias=bs_sb[:, dc:dc + 1])
            uo_sb = wpool.tile([P, TQ], F32, tag="uo")
            nc.vector.tensor_mul(uo_sb[:], audio_sb[:, dc, :].bitcast(F32), gs_sb[:])
            nc.sync.dma_start(uoutT[:, dc, :], uo_sb[:])

    nc.compile()
    return nc


def _chunk_pd(x):
    """[D, F] -> [P, NC, F] with [p, c, f] = x[c*P + p, f]."""
    f = x.shape[1]
    return np.ascontiguousarray(
        x.reshape(NC, P, f).transpose(1, 0, 2), dtype=np.float32
    )


def _chunk_vec(x):
    """[D] -> [P, NC] with [p, c] = x[c*P + p]."""
    return np.ascontiguousarray(x.reshape(NC, P).T, dtype=np.float32)


def kernel(audio_emb, text_emb, audio_len, text_len,
           W_attn, b_attn, v, W_u, b_u, W_s, b_s):
    global _cached_nc, LAST_EXEC_NS
    audio_emb = np.asarray(audio_emb, dtype=np.float32)
    text_emb = np.asarray(text_emb, dtype=np.float32)
    audio_len = np.asarray(audio_len)
    text_len = np.asarray(text_len)
    W_attn = np.asarray(W_attn, dtype=np.float32)
    b_attn = np.asarray(b_attn, dtype=np.float32)
    v = np.asarray(v, dtype=np.float32)
    W_u = np.asarray(W_u, dtype=np.float32)
    b_u = np.asarray(b_u, dtype=np.float32)
    W_s = np.asarray(W_s, dtype=np.float32)
    b_s = np.asarray(b_s, dtype=np.float32)

    wq3 = _chunk_pd(W_attn[:, :D].T)
    wkv3 = _chunk_pd(W_attn[:, D:].T)
    wu3 = _chunk_pd(W_u.T)
    ws3 = _chunk_pd(W_s.T)
    battn_c = _chunk_vec(b_attn)
    bu_c = _chunk_vec(b_u)
    bs_c = _chunk_vec(b_s)
    v_c = np.zeros((P, NC, 2), dtype=np.float32)
    v_c[:, :, 0] = _chunk_vec(v)

    q_ar = np.arange(TQ)
    k_ar = np.arange(TK)
    in_maps = []
    for b in range(B):
        valid = (q_ar[:, None] < int(audio_len[b])) & (k_ar[None, :] < int(text_len[b]))
        mask = np.where(valid, np.float32(0.0), np.float32(NEG)).astype(np.float32)
        in_maps.append({
            "audio3": _chunk_pd(audio_emb[b].T),
            "wq3": wq3,
            "wkv3": wkv3,
            "wu3": wu3,
            "ws3": ws3,
            "text2": np.ascontiguousarray(text_emb[b], dtype=np.float32),
            "text3": np.ascontiguousarray(
                text_emb[b].T.reshape(NC, P, TK).transpose(1, 0, 2), dtype=np.float32
            ),
            "battn_c": battn_c,
            "bu_c": bu_c,
            "bs_c": bs_c,
            "v_c": v_c,
            "mask3": np.ascontiguousarray(
                mask.reshape(NC, P, TK).transpose(1, 0, 2), dtype=np.float32
            ),
        })

    if _cached_nc is None:
        _cached_nc = _build()
    res = run_bass_kernel_spmd(_cached_nc, in_maps, list(range(B)), trace=TRACE)
    LAST_EXEC_NS = res.exec_time_ns

    u_out = np.empty((B, TQ, D), dtype=np.float32)
    s_out = np.empty((B, TQ, D), dtype=np.float32)
    for b in range(B):
        uT = res.results[b]["uoutT"].transpose(1, 0, 2).reshape(D, TQ)
        sT = res.results[b]["soutT"].transpose(1, 0, 2).reshape(D, TQ)
        u_out[b] = uT.T
        s_out[b] = sT.T
    return (u_out, s_out)


# revision 8
# speedup vs baseline: 1.5935x; 1.5935x over previous
"""Trainium2 Bass kernel for nn_CrossAttentionGating.

Sharding: data-parallel over batch B=8 across 8 cores (1 batch element per
core); all weights replicated. Host numpy does all layout prep (transposes,
chunking into 128-partition tiles, additive attention mask from lengths).

Per-core device pipeline (D=512 split into 4 chunks of 128 partitions):
  qp_T[d,q] = Wq^T.T @ audio^T          (PE, f32r)
  kp_T[d,k] = Wkv^T.T @ text^T + b_attn (PE + ACT bias)
  for each k:  X = qp_T + kp_T[:,k]     (DVE tensor_scalar per-partition add)
               H = tanh(X)              (ACT, batched big instructions)
               score_T[k,:] += v_c.T@H  (PE, M=1 f32r matmuls, PSUM accum)
  softmax over k: PE-transpose score to [q,k], +mask, max/exp/sum/recip
  ctx_T[e,q] = text.T @ attn_T          (PE)
  g_u = sigmoid(Wu^T.T @ audio^T + b_u); s_out_T = ctx_T * g_u
  g_s = sigmoid(Ws^T.T @ ctx_T + b_s);   u_out_T = audio_T * g_s
"""

import sys

for _p in ("/opt/trn_rl_repo", "/opt/pypackages"):
    if _p not in sys.path:
        sys.path.append(_p)

from contextlib import ExitStack

import ml_dtypes
import numpy as np

import concourse.bacc as bacc
import concourse.tile as tile
import concourse.mybir as mybir
from concourse import masks
from concourse.bass_utils import run_bass_kernel_spmd

B, TQ, TK, D = 8, 512, 64, 512
P = 128
NC = D // P  # 4 chunks of the embedding dim
KB = 4      # k's per tanh batch
NEG = -1e10
F32 = mybir.dt.float32
F32R = mybir.dt.float32r
BF16 = mybir.dt.bfloat16
FP16 = mybir.dt.float16
AF = mybir.ActivationFunctionType

TRACE = False
LAST_EXEC_NS = None

_cached_nc = None


def _build():
    nc = bacc.Bacc("TRN2", target_bir_lowering=False, debug=False, num_devices=B)

    audio3 = nc.dram_tensor("audio3", [P, NC, TQ], F32R, kind="ExternalInput")
    wq3 = nc.dram_tensor("wq3", [P, NC, D], F32R, kind="ExternalInput")
    wkv3 = nc.dram_tensor("wkv3", [P, NC, D], F32R, kind="ExternalInput")
    wu3 = nc.dram_tensor("wu3", [P, NC, D], F32R, kind="ExternalInput")
    ws3 = nc.dram_tensor("ws3", [P, NC, D], F32R, kind="ExternalInput")
    text2 = nc.dram_tensor("text2", [TK, D], F32R, kind="ExternalInput")
    text3 = nc.dram_tensor("text3", [P, NC, TK], F32R, kind="ExternalInput")
    battn_c = nc.dram_tensor("battn_c", [P, NC], F32, kind="ExternalInput")
    bu_c = nc.dram_tensor("bu_c", [P, NC], F32, kind="ExternalInput")
    bs_c = nc.dram_tensor("bs_c", [P, NC], F32, kind="ExternalInput")
    v_c = nc.dram_tensor("v_c", [P, NC], FP16, kind="ExternalInput")
    mask3 = nc.dram_tensor("mask3", [P, NC, TK], F32, kind="ExternalInput")
    uoutT = nc.dram_tensor("uoutT", [P, NC, TQ], F32, kind="ExternalOutput")
    soutT = nc.dram_tensor("soutT", [P, NC, TQ], F32, kind="ExternalOutput")

    with tile.TileContext(nc) as tc, ExitStack() as ctx:
        cpool = ctx.enter_context(tc.tile_pool(name="const", bufs=1))
        ppool = ctx.enter_context(tc.tile_pool(name="ps", bufs=4, space="PSUM"))
        spool = ctx.enter_context(tc.tile_pool(name="score", bufs=1, space="PSUM"))
        xpool = ctx.enter_context(tc.tile_pool(name="x", bufs=2))
        hpool = ctx.enter_context(tc.tile_pool(name="h", bufs=2))
        wpool = ctx.enter_context(tc.tile_pool(name="work", bufs=2))

        # ---- persistent loads ----
        audio_sb = cpool.tile([P, NC, TQ], F32R)
        nc.sync.dma_start(audio_sb[:], audio3[:])
        wq_sb = cpool.tile([P, NC, D], F32R)
        nc.sync.dma_start(wq_sb[:], wq3[:])
        wkv_sb = cpool.tile([P, NC, D], F32R)
        nc.sync.dma_start(wkv_sb[:], wkv3[:])
        wu_sb = cpool.tile([P, NC, D], F32R)
        nc.sync.dma_start(wu_sb[:], wu3[:])
        ws_sb = cpool.tile([P, NC, D], F32R)
        nc.sync.dma_start(ws_sb[:], ws3[:])
        text_sb = cpool.tile([TK, D], F32R)
        nc.sync.dma_start(text_sb[:], text2[:])
        text3_sb = cpool.tile([P, NC, TK], F32R)
        nc.sync.dma_start(text3_sb[:], text3[:])
        battn_sb = cpool.tile([P, NC], F32)
        nc.sync.dma_start(battn_sb[:], battn_c[:])
        bu_sb = cpool.tile([P, NC], F32)
        nc.sync.dma_start(bu_sb[:], bu_c[:])
        bs_sb = cpool.tile([P, NC], F32)
        nc.sync.dma_start(bs_sb[:], bs_c[:])
        v_sb = cpool.tile([P, NC], FP16)
        nc.sync.dma_start(v_sb[:], v_c[:])
        mask_sb = cpool.tile([P, NC, TK], F32)
        nc.sync.dma_start(mask_sb[:], mask3[:])

        ident = cpool.tile([P, P], F32)
        masks.make_identity(nc, ident[:])

        # ---- projections ----
        kp_sb = cpool.tile([P, NC, TK], F32)
        for dc in range(NC):
            kp_ps = ppool.tile([P, TK], F32, tag="ps")
            for ec in range(NC):
                nc.tensor.matmul(
                    kp_ps[:],
                    wkv_sb[:, ec, dc * P:(dc + 1) * P],
                    text3_sb[:, ec, :],
                    start=(ec == 0),
                    stop=(ec == NC - 1),
                )
            nc.scalar.activation(
                kp_sb[:, dc, :], kp_ps[:], AF.Identity, bias=battn_sb[:, dc:dc + 1]
            )

        qp_sb = cpool.tile([P, NC, TQ], F32)
        for dc in range(NC):
            qp_ps = ppool.tile([P, TQ], F32, tag="ps")
            for ec in range(NC):
                nc.tensor.matmul(
                    qp_ps[:],
                    wq_sb[:, ec, dc * P:(dc + 1) * P],
                    audio_sb[:, ec, :],
                    start=(ec == 0),
                    stop=(ec == NC - 1),
                )
            nc.vector.tensor_copy(qp_sb[:, dc, :], qp_ps[:])

        # ---- scores: score[q, k] = v . tanh(qp_T[:,q] + kp_T[:,k]) ----
        # lhsT = H chunk [128d, 128q] in bf16 (1 cyc/col weight load), rhs =
        # v chunk [128,1] bf16; accumulates [128q, 1] per (k, qc) over the 4
        # d-chunks directly into the [q, k]-layout PSUM score bank.
        score_ps = spool.tile([P, NC, TK], F32)
        for kb in range(TK // KB):
            x_t = xpool.tile([P, KB, NC, TQ], F32, tag="x")
            for kk in range(KB):
                k = kb * KB + kk
                for dc in range(NC):
                    nc.vector.tensor_scalar_add(
                        x_t[:, kk, dc, :], qp_sb[:, dc, :], kp_sb[:, dc, k:k + 1]
                    )
            h_t = hpool.tile([P, KB, NC, TQ], FP16, tag="h")
            nc.scalar.activation(h_t[:], x_t[:], AF.Tanh)
            for kk in range(KB):
                k = kb * KB + kk
                for qc in range(NC):
                    for dc in range(NC):
                        nc.tensor.matmul(
                            score_ps[:, qc, k:k + 1],
                            h_t[:, kk, dc, qc * P:(qc + 1) * P],
                            v_sb[:, dc:dc + 1],
                            start=(dc == 0),
                            stop=(dc == NC - 1),
                        )

        # ---- softmax over k (already in [q, k] layout) ----
        sm_sb = cpool.tile([P, NC, TK], F32)
        e_sb = cpool.tile([P, NC, TK], F32)
        attn_sb = cpool.tile([P, NC, TK], F32)
        attnT_sb = cpool.tile([TK, TQ], F32R)
        for qc in range(NC):
            nc.vector.tensor_add(sm_sb[:, qc, :], score_ps[:, qc, :], mask_sb[:, qc, :])
            nmax = wpool.tile([P, 1], F32, tag="nmax")
            nc.vector.reduce_max(
                nmax[:], sm_sb[:, qc, :], axis=mybir.AxisListType.X, negate=True
            )
            nc.scalar.activation(e_sb[:, qc, :], sm_sb[:, qc, :], AF.Exp, bias=nmax[:])
            ssum = wpool.tile([P, 1], F32, tag="ssum")
            nc.vector.reduce_sum(ssum[:], e_sb[:, qc, :], axis=mybir.AxisListType.X)
            rinv = wpool.tile([P, 1], F32, tag="rinv")
            nc.vector.reciprocal(rinv[:], ssum[:])
            nc.vector.tensor_scalar_mul(attn_sb[:, qc, :], e_sb[:, qc, :], rinv[:])
            at_ps = ppool.tile([TK, P], F32, tag="ps")
            nc.tensor.transpose(at_ps[:], attn_sb[:, qc, :], ident[:])
            nc.vector.tensor_copy(attnT_sb[:, qc * P:(qc + 1) * P], at_ps[:])

        # ---- ctx_T[e, q] = text.T @ attn_T ----
        ctx_sb = cpool.tile([P, NC, TQ], F32R)
        for ec in range(NC):
            ctx_ps = ppool.tile([P, TQ], F32, tag="ps")
            nc.tensor.matmul(
                ctx_ps[:],
                text_sb[:, ec * P:(ec + 1) * P],
                attnT_sb[:],
                start=True,
                stop=True,
            )
            nc.vector.tensor_copy(ctx_sb[:, ec, :], ctx_ps[:])

        # ---- gating ----
        for dc in range(NC):
            gu_ps = ppool.tile([P, TQ], F32, tag="ps")
            for ec in range(NC):
                nc.tensor.matmul(
                    gu_ps[:],
                    wu_sb[:, ec, dc * P:(dc + 1) * P],
                    audio_sb[:, ec, :],
                    start=(ec == 0),
                    stop=(ec == NC - 1),
                )
            gu_sb = wpool.tile([P, TQ], F32, tag="gu")
            nc.scalar.activation(gu_sb[:], gu_ps[:], AF.Sigmoid, bias=bu_sb[:, dc:dc + 1])
            so_sb = wpool.tile([P, TQ], F32, tag="so")
            nc.vector.tensor_mul(so_sb[:], ctx_sb[:, dc, :].bitcast(F32), gu_sb[:])
            nc.sync.dma_start(soutT[:, dc, :], so_sb[:])

        for dc in range(NC):
            gs_ps = ppool.tile([P, TQ], F32, tag="ps")
            for ec in range(NC):
                nc.tensor.matmul(
                    gs_ps[:],
                    ws_sb[:, ec, dc * P:(dc + 1) * P],
                    ctx_sb[:, ec, :],
                    start=(ec == 0),
                    stop=(ec == NC - 1),
                )
            gs_sb = wpool.tile([P, TQ], F32, tag="gs")
            nc.scalar.activation(gs_sb[:], gs_ps[:], AF.Sigmoid, bias=bs_sb[:, dc:dc + 1])
            uo_sb = wpool.tile([P, TQ], F32, tag="uo")
            nc.vector.tensor_mul(uo_sb[:], audio_sb[:, dc, :].bitcast(F32), gs_sb[:])
            nc.sync.dma_start(uoutT[:, dc, :], uo_sb[:])

    nc.compile()
    return nc


def _chunk_pd(x):
    """[D, F] -> [P, NC, F] with [p, c, f] = x[c*P + p, f]."""
    f = x.shape[1]
    return np.ascontiguousarray(
        x.reshape(NC, P, f).transpose(1, 0, 2), dtype=np.float32
    )


def _chunk_vec(x):
    """[D] -> [P, NC] with [p, c] = x[c*P + p]."""
    return np.ascontiguousarray(x.reshape(NC, P).T, dtype=np.float32)


def kernel(audio_emb, text_emb, audio_len, text_len,
           W_attn, b_attn, v, W_u, b_u, W_s, b_s):
    global _cached_nc, LAST_EXEC_NS
    audio_emb = np.asarray(audio_emb, dtype=np.float32)
    text_emb = np.asarray(text_emb, dtype=np.float32)
    audio_len = np.asarray(audio_len)
    text_len = np.asarray(text_len)
    W_attn = np.asarray(W_attn, dtype=np.float32)
    b_attn = np.asarray(b_attn, dtype=np.float32)
    v = np.asarray(v, dtype=np.float32)
    W_u = np.asarray(W_u, dtype=np.float32)
    b_u = np.asarray(b_u, dtype=np.float32)
    W_s = np.asarray(W_s, dtype=np.float32)
    b_s = np.asarray(b_s, dtype=np.float32)

    wq3 = _chunk_pd(W_attn[:, :D].T)
    wkv3 = _chunk_pd(W_attn[:, D:].T)
    wu3 = _chunk_pd(W_u.T)
    ws3 = _chunk_pd(W_s.T)
    battn_c = _chunk_vec(b_attn)
    bu_c = _chunk_vec(b_u)
    bs_c = _chunk_vec(b_s)
    v_c = _chunk_vec(v).astype(np.float16)

    q_ar = np.arange(TQ)
    k_ar = np.arange(TK)
    in_maps = []
    for b in range(B):
        valid = (q_ar[:, None] < int(audio_len[b])) & (k_ar[None, :] < int(text_len[b]))
        mask = np.where(valid, np.float32(0.0), np.float32(NEG)).astype(np.float32)
        in_maps.append({
            "audio3": _chunk_pd(audio_emb[b].T),
            "wq3": wq3,
            "wkv3": wkv3,
            "wu3": wu3,
            "ws3": ws3,
            "text2": np.ascontiguousarray(text_emb[b], dtype=np.float32),
            "text3": np.ascontiguousarray(
                text_emb[b].T.reshape(NC, P, TK).transpose(1, 0, 2), dtype=np.float32
            ),
            "battn_c": battn_c,
            "bu_c": bu_c,
            "bs_c": bs_c,
            "v_c": v_c,
            "mask3": np.ascontiguousarray(
                mask.reshape(NC, P, TK).transpose(1, 0, 2), dtype=np.float32
            ),
        })

    if _cached_nc is None:
        _cached_nc = _build()
    res = run_bass_kernel_spmd(_cached_nc, in_maps, list(range(B)), trace=TRACE)
    LAST_EXEC_NS = res.exec_time_ns

    u_out = np.empty((B, TQ, D), dtype=np.float32)
    s_out = np.empty((B, TQ, D), dtype=np.float32)
    for b in range(B):
        uT = res.results[b]["uoutT"].transpose(1, 0, 2).reshape(D, TQ)
        sT = res.results[b]["soutT"].transpose(1, 0, 2).reshape(D, TQ)
        u_out[b] = uT.T
        s_out[b] = sT.T
    return (u_out, s_out)


# revision 9
# speedup vs baseline: 1.6005x; 1.0044x over previous
"""Trainium2 Bass kernel for nn_CrossAttentionGating.

Sharding: data-parallel over batch B=8 across 8 cores (1 batch element per
core); all weights replicated. Host numpy does all layout prep (transposes,
chunking into 128-partition tiles, additive attention mask from lengths).

Per-core device pipeline (D=512 split into 4 chunks of 128 partitions):
  qp_T[d,q] = Wq^T.T @ audio^T          (PE, f32r)
  kp_T[d,k] = Wkv^T.T @ text^T + b_attn (PE + ACT bias)
  for each k:  X = qp_T + kp_T[:,k]     (DVE tensor_scalar per-partition add)
               H = tanh(X)              (ACT, batched big instructions)
               score_T[k,:] += v_c.T@H  (PE, M=1 f32r matmuls, PSUM accum)
  softmax over k: PE-transpose score to [q,k], +mask, max/exp/sum/recip
  ctx_T[e,q] = text.T @ attn_T          (PE)
  g_u = sigmoid(Wu^T.T @ audio^T + b_u); s_out_T = ctx_T * g_u
  g_s = sigmoid(Ws^T.T @ ctx_T + b_s);   u_out_T = audio_T * g_s
"""

import sys

for _p in ("/opt/trn_rl_repo", "/opt/pypackages"):
    if _p not in sys.path:
        sys.path.append(_p)

from contextlib import ExitStack

import ml_dtypes
import numpy as np

import concourse.bacc as bacc
import concourse.tile as tile
import concourse.mybir as mybir
from concourse import masks
from concourse.bass_utils import run_bass_kernel_spmd

B, TQ, TK, D = 8, 512, 64, 512
P = 128
NC = D // P  # 4 chunks of the embedding dim
KB = 4      # k's per tanh batch
NEG = -1e10
F32 = mybir.dt.float32
F32R = mybir.dt.float32r
BF16 = mybir.dt.bfloat16
FP16 = mybir.dt.float16
AF = mybir.ActivationFunctionType

TRACE = False
LAST_EXEC_NS = None

_cached_nc = None


def _build():
    nc = bacc.Bacc("TRN2", target_bir_lowering=False, debug=False, num_devices=B)

    audio3 = nc.dram_tensor("audio3", [P, NC, TQ], F32R, kind="ExternalInput")
    wq3 = nc.dram_tensor("wq3", [P, NC, D], F32R, kind="ExternalInput")
    wkv3 = nc.dram_tensor("wkv3", [P, NC, D], F32R, kind="ExternalInput")
    wu3 = nc.dram_tensor("wu3", [P, NC, D], F32R, kind="ExternalInput")
    ws3 = nc.dram_tensor("ws3", [P, NC, D], F32R, kind="ExternalInput")
    text2 = nc.dram_tensor("text2", [TK, D], F32R, kind="ExternalInput")
    text3 = nc.dram_tensor("text3", [P, NC, TK], F32R, kind="ExternalInput")
    battn_c = nc.dram_tensor("battn_c", [P, NC], F32, kind="ExternalInput")
    bu_c = nc.dram_tensor("bu_c", [P, NC], F32, kind="ExternalInput")
    bs_c = nc.dram_tensor("bs_c", [P, NC], F32, kind="ExternalInput")
    v_c = nc.dram_tensor("v_c", [P, NC], FP16, kind="ExternalInput")
    mask3 = nc.dram_tensor("mask3", [P, NC, TK], F32, kind="ExternalInput")
    uoutT = nc.dram_tensor("uoutT", [P, NC, TQ], F32, kind="ExternalOutput")
    soutT = nc.dram_tensor("soutT", [P, NC, TQ], F32, kind="ExternalOutput")

    with tile.TileContext(nc) as tc, ExitStack() as ctx:
        cpool = ctx.enter_context(tc.tile_pool(name="const", bufs=1))
        ppool = ctx.enter_context(tc.tile_pool(name="ps", bufs=4, space="PSUM"))
        spool = ctx.enter_context(tc.tile_pool(name="score", bufs=1, space="PSUM"))
        xpool = ctx.enter_context(tc.tile_pool(name="x", bufs=3))
        hpool = ctx.enter_context(tc.tile_pool(name="h", bufs=3))
        wpool = ctx.enter_context(tc.tile_pool(name="work", bufs=2))

        # ---- persistent loads ----
        audio_sb = cpool.tile([P, NC, TQ], F32R)
        nc.sync.dma_start(audio_sb[:], audio3[:])
        wq_sb = cpool.tile([P, NC, D], F32R)
        nc.sync.dma_start(wq_sb[:], wq3[:])
        wkv_sb = cpool.tile([P, NC, D], F32R)
        nc.sync.dma_start(wkv_sb[:], wkv3[:])
        wu_sb = cpool.tile([P, NC, D], F32R)
        nc.sync.dma_start(wu_sb[:], wu3[:])
        ws_sb = cpool.tile([P, NC, D], F32R)
        nc.sync.dma_start(ws_sb[:], ws3[:])
        text_sb = cpool.tile([TK, D], F32R)
        nc.sync.dma_start(text_sb[:], text2[:])
        text3_sb = cpool.tile([P, NC, TK], F32R)
        nc.sync.dma_start(text3_sb[:], text3[:])
        battn_sb = cpool.tile([P, NC], F32)
        nc.sync.dma_start(battn_sb[:], battn_c[:])
        bu_sb = cpool.tile([P, NC], F32)
        nc.sync.dma_start(bu_sb[:], bu_c[:])
        bs_sb = cpool.tile([P, NC], F32)
        nc.sync.dma_start(bs_sb[:], bs_c[:])
        v_sb = cpool.tile([P, NC], FP16)
        nc.sync.dma_start(v_sb[:], v_c[:])
        mask_sb = cpool.tile([P, NC, TK], F32)
        nc.sync.dma_start(mask_sb[:], mask3[:])

        ident = cpool.tile([P, P], F32)
        masks.make_identity(nc, ident[:])

        # ---- projections ----
        kp_sb = cpool.tile([P, NC, TK], F32)
        for dc in range(NC):
            kp_ps = ppool.tile([P, TK], F32, tag="ps")
            for ec in range(NC):
                nc.tensor.matmul(
                    kp_ps[:],
                    wkv_sb[:, ec, dc * P:(dc + 1) * P],
                    text3_sb[:, ec, :],
                    start=(ec == 0),
                    stop=(ec == NC - 1),
                )
            nc.scalar.activation(
                kp_sb[:, dc, :], kp_ps[:], AF.Identity, bias=battn_sb[:, dc:dc + 1]
            )

        qp_sb = cpool.tile([P, NC, TQ], FP16)
        for dc in range(NC):
            qp_ps = ppool.tile([P, TQ], F32, tag="ps")
            for ec in range(NC):
                nc.tensor.matmul(
                    qp_ps[:],
                    wq_sb[:, ec, dc * P:(dc + 1) * P],
                    audio_sb[:, ec, :],
                    start=(ec == 0),
                    stop=(ec == NC - 1),
                )
            nc.vector.tensor_copy(qp_sb[:, dc, :], qp_ps[:])

        # ---- scores: score[q, k] = v . tanh(qp_T[:,q] + kp_T[:,k]) ----
        # lhsT = H chunk [128d, 128q] in bf16 (1 cyc/col weight load), rhs =
        # v chunk [128,1] bf16; accumulates [128q, 1] per (k, qc) over the 4
        # d-chunks directly into the [q, k]-layout PSUM score bank.
        score_ps = spool.tile([P, NC, TK], F32)
        for kb in range(TK // KB):
            x_t = xpool.tile([P, KB, NC, TQ], FP16, tag="x")
            for kk in range(KB):
                k = kb * KB + kk
                for dc in range(NC):
                    nc.vector.tensor_scalar_add(
                        x_t[:, kk, dc, :], qp_sb[:, dc, :], kp_sb[:, dc, k:k + 1]
                    )
            h_t = hpool.tile([P, KB, NC, TQ], FP16, tag="h")
            nc.scalar.activation(h_t[:], x_t[:], AF.Tanh)
            for kk in range(KB):
                k = kb * KB + kk
                for qc in range(NC):
                    for dc in range(NC):
                        nc.tensor.matmul(
                            score_ps[:, qc, k:k + 1],
                            h_t[:, kk, dc, qc * P:(qc + 1) * P],
                            v_sb[:, dc:dc + 1],
                            start=(dc == 0),
                            stop=(dc == NC - 1),
                        )

        # ---- softmax over k (already in [q, k] layout) ----
        sm_sb = cpool.tile([P, NC, TK], F32)
        e_sb = cpool.tile([P, NC, TK], F32)
        attn_sb = cpool.tile([P, NC, TK], F32)
        attnT_sb = cpool.tile([TK, TQ], F32R)
        for qc in range(NC):
            nc.vector.tensor_add(sm_sb[:, qc, :], score_ps[:, qc, :], mask_sb[:, qc, :])
            nmax = wpool.tile([P, 1], F32, tag="nmax")
            nc.vector.reduce_max(
                nmax[:], sm_sb[:, qc, :], axis=mybir.AxisListType.X, negate=True
            )
            nc.scalar.activation(e_sb[:, qc, :], sm_sb[:, qc, :], AF.Exp, bias=nmax[:])
            ssum = wpool.tile([P, 1], F32, tag="ssum")
            nc.vector.reduce_sum(ssum[:], e_sb[:, qc, :], axis=mybir.AxisListType.X)
            rinv = wpool.tile([P, 1], F32, tag="rinv")
            nc.vector.reciprocal(rinv[:], ssum[:])
            nc.vector.tensor_scalar_mul(attn_sb[:, qc, :], e_sb[:, qc, :], rinv[:])
            at_ps = ppool.tile([TK, P], F32, tag="ps")
            nc.tensor.transpose(at_ps[:], attn_sb[:, qc, :], ident[:])
            nc.vector.tensor_copy(attnT_sb[:, qc * P:(qc + 1) * P], at_ps[:])

        # ---- ctx_T[e, q] = text.T @ attn_T ----
        ctx_sb = cpool.tile([P, NC, TQ], F32R)
        for ec in range(NC):
            ctx_ps = ppool.tile([P, TQ], F32, tag="ps")
            nc.tensor.matmul(
                ctx_ps[:],
                text_sb[:, ec * P:(ec + 1) * P],
                attnT_sb[:],
                start=True,
                stop=True,
            )
            nc.vector.tensor_copy(ctx_sb[:, ec, :], ctx_ps[:])

        # ---- gating ----
        for dc in range(NC):
            gu_ps = ppool.tile([P, TQ], F32, tag="ps")
            for ec in range(NC):
                nc.tensor.matmul(
                    gu_ps[:],
                    wu_sb[:, ec, dc * P:(dc + 1) * P],
                    audio_sb[:, ec, :],
                    start=(ec == 0),
                    stop=(ec == NC - 1),
                )
            gu_sb = wpool.tile([P, TQ], F32, tag="gu")
            nc.scalar.activation(gu_sb[:], gu_ps[:], AF.Sigmoid, bias=bu_sb[:, dc:dc + 1])
            so_sb = wpool.tile([P, TQ], F32, tag="so")
            nc.vector.tensor_mul(so_sb[:], ctx_sb[:, dc, :].bitcast(F32), gu_sb[:])
            nc.sync.dma_start(soutT[:, dc, :], so_sb[:])

        for dc in range(NC):
            gs_ps = ppool.tile([P, TQ], F32, tag="ps")
            for ec in range(NC):
                nc.tensor.matmul(
                    gs_ps[:],
                    ws_sb[:, ec, dc * P:(dc + 1) * P],
                    ctx_sb[:, ec, :],
                    start=(ec == 0),
                    stop=(ec == NC - 1),
                )
            gs_sb = wpool.tile([P, TQ], F32, tag="gs")
            nc.scalar.activation(gs_sb[:], gs_ps[:], AF.Sigmoid, bias=bs_sb[:, dc:dc + 1])
            uo_sb = wpool.tile([P, TQ], F32, tag="uo")
            nc.vector.tensor_mul(uo_sb[:], audio_sb[:, dc, :].bitcast(F32), gs_sb[:])
            nc.sync.dma_start(uoutT[:, dc, :], uo_sb[:])

    nc.compile()
    return nc


def _chunk_pd(x):
    """[D, F] -> [P, NC, F] with [p, c, f] = x[c*P + p, f]."""
    f = x.shape[1]
    return np.ascontiguousarray(
        x.reshape(NC, P, f).transpose(1, 0, 2), dtype=np.float32
    )


def _chunk_vec(x):
    """[D] -> [P, NC] with [p, c] = x[c*P + p]."""
    return np.ascontiguousarray(x.reshape(NC, P).T, dtype=np.float32)


def kernel(audio_emb, text_emb, audio_len, text_len,
           W_attn, b_attn, v, W_u, b_u, W_s, b_s):
    global _cached_nc, LAST_EXEC_NS
    audio_emb = np.asarray(audio_emb, dtype=np.float32)
    text_emb = np.asarray(text_emb, dtype=np.float32)
    audio_len = np.asarray(audio_len)
    text_len = np.asarray(text_len)
    W_attn = np.asarray(W_attn, dtype=np.float32)
    b_attn = np.asarray(b_attn, dtype=np.float32)
    v = np.asarray(v, dtype=np.float32)
    W_u = np.asarray(W_u, dtype=np.float32)
    b_u = np.asarray(b_u, dtype=np.float32)
    W_s = np.asarray(W_s, dtype=np.float32)
    b_s = np.asarray(b_s, dtype=np.float32)

    wq3 = _chunk_pd(W_attn[:, :D].T)
    wkv3 = _chunk_pd(W_attn[:, D:].T)
    wu3 = _chunk_pd(W_u.T)
    ws3 = _chunk_pd(W_s.T)
    battn_c = _chunk_vec(b_attn)
    bu_c = _chunk_vec(b_u)
    bs_c = _chunk_vec(b_s)
    v_c = _chunk_vec(v).astype(np.float16)

    q_ar = np.arange(TQ)
    k_ar = np.arange(TK)
    in_maps = []
    for b in range(B):
        valid = (q_ar[:, None] < int(audio_len[b])) & (k_ar[None, :] < int(text_len[b]))
        mask = np.where(valid, np.float32(0.0), np.float32(NEG)).astype(np.float32)
        in_maps.append({
            "audio3": _chunk_pd(audio_emb[b].T),
            "wq3": wq3,
            "wkv3": wkv3,
            "wu3": wu3,
            "ws3": ws3,
            "text2": np.ascontiguousarray(text_emb[b], dtype=np.float32),
            "text3": np.ascontiguousarray(
                text_emb[b].T.reshape(NC, P, TK).transpose(1, 0, 2), dtype=np.float32
            ),
            "battn_c": battn_c,
            "bu_c": bu_c,
            "bs_c": bs_c,
            "v_c": v_c,
            "mask3": np.ascontiguousarray(
                mask.reshape(NC, P, TK).transpose(1, 0, 2), dtype=np.float32
            ),
        })

    if _cached_nc is None:
        _cached_nc = _build()
    res = run_bass_kernel_spmd(_cached_nc, in_maps, list(range(B)), trace=TRACE)
    LAST_EXEC_NS = res.exec_time_ns

    u_out = np.empty((B, TQ, D), dtype=np.float32)
    s_out = np.empty((B, TQ, D), dtype=np.float32)
    for b in range(B):
        uT = res.results[b]["uoutT"].transpose(1, 0, 2).reshape(D, TQ)
        sT = res.results[b]["soutT"].transpose(1, 0, 2).reshape(D, TQ)
        u_out[b] = uT.T
        s_out[b] = sT.T
    return (u_out, s_out)


# revision 11
# speedup vs baseline: 1.6305x; 1.0187x over previous
"""Trainium2 Bass kernel for nn_CrossAttentionGating.

Sharding: data-parallel over batch B=8 across 8 cores (1 batch element per
core); all weights replicated. Host numpy does all layout prep (transposes,
chunking into 128-partition tiles, additive attention mask from lengths).

Per-core device pipeline (D=512 split into 4 chunks of 128 partitions):
  qp_T[d,q] = Wq^T.T @ audio^T          (PE, f32r)
  kp_T[d,k] = Wkv^T.T @ text^T + b_attn (PE + ACT bias)
  for each k:  X = qp_T + kp_T[:,k]     (DVE tensor_scalar per-partition add)
               H = tanh(X)              (ACT, batched big instructions)
               score_T[k,:] += v_c.T@H  (PE, M=1 f32r matmuls, PSUM accum)
  softmax over k: PE-transpose score to [q,k], +mask, max/exp/sum/recip
  ctx_T[e,q] = text.T @ attn_T          (PE)
  g_u = sigmoid(Wu^T.T @ audio^T + b_u); s_out_T = ctx_T * g_u
  g_s = sigmoid(Ws^T.T @ ctx_T + b_s);   u_out_T = audio_T * g_s
"""

import sys

for _p in ("/opt/trn_rl_repo", "/opt/pypackages"):
    if _p not in sys.path:
        sys.path.append(_p)

from contextlib import ExitStack

import ml_dtypes
import numpy as np

import concourse.bacc as bacc
import concourse.tile as tile
import concourse.mybir as mybir
from concourse import masks
from concourse.bass_utils import run_bass_kernel_spmd

B, TQ, TK, D = 8, 512, 64, 512
P = 128
NC = D // P  # 4 chunks of the embedding dim
KB = 4      # k's per tanh batch
NEG = -1e10
F32 = mybir.dt.float32
F32R = mybir.dt.float32r
BF16 = mybir.dt.bfloat16
FP16 = mybir.dt.float16
AF = mybir.ActivationFunctionType

TRACE = False
LAST_EXEC_NS = None

_cached_nc = None


def _build():
    nc = bacc.Bacc("TRN2", target_bir_lowering=False, debug=False, num_devices=B)

    audio3 = nc.dram_tensor("audio3", [P, NC, TQ], F32R, kind="ExternalInput")
    wq3 = nc.dram_tensor("wq3", [P, NC, D], F32R, kind="ExternalInput")
    wkv3 = nc.dram_tensor("wkv3", [P, NC, D], F32R, kind="ExternalInput")
    wu3 = nc.dram_tensor("wu3", [P, NC, D], F32R, kind="ExternalInput")
    ws3 = nc.dram_tensor("ws3", [P, NC, D], F32R, kind="ExternalInput")
    text2 = nc.dram_tensor("text2", [TK, D], F32R, kind="ExternalInput")
    text3 = nc.dram_tensor("text3", [P, NC, TK], F32R, kind="ExternalInput")
    battn_c = nc.dram_tensor("battn_c", [P, NC], F32, kind="ExternalInput")
    bu_c = nc.dram_tensor("bu_c", [P, NC], F32, kind="ExternalInput")
    bs_c = nc.dram_tensor("bs_c", [P, NC], F32, kind="ExternalInput")
    v_c = nc.dram_tensor("v_c", [P, NC], FP16, kind="ExternalInput")
    mask3 = nc.dram_tensor("mask3", [P, NC, TK], F32, kind="ExternalInput")
    uoutT = nc.dram_tensor("uoutT", [P, NC, TQ], F32, kind="ExternalOutput")
    soutT = nc.dram_tensor("soutT", [P, NC, TQ], F32, kind="ExternalOutput")

    with tile.TileContext(nc) as tc, ExitStack() as ctx:
        cpool = ctx.enter_context(tc.tile_pool(name="const", bufs=1))
        ppool = ctx.enter_context(tc.tile_pool(name="ps", bufs=4, space="PSUM"))
        spool = ctx.enter_context(tc.tile_pool(name="score", bufs=1, space="PSUM"))
        xpool = ctx.enter_context(tc.tile_pool(name="x", bufs=3))
        hpool = ctx.enter_context(tc.tile_pool(name="h", bufs=3))
        wpool = ctx.enter_context(tc.tile_pool(name="work", bufs=2))

        # ---- persistent loads (spread across per-engine DMA queues) ----
        audio_sb = cpool.tile([P, NC, TQ], F32R)
        wq_sb = cpool.tile([P, NC, D], F32R)
        wkv_sb = cpool.tile([P, NC, D], F32R)
        wu_sb = cpool.tile([P, NC, D], F32R)
        ws_sb = cpool.tile([P, NC, D], F32R)
        text_sb = cpool.tile([TK, D], F32R)
        text3_sb = cpool.tile([P, NC, TK], F32R)
        battn_sb = cpool.tile([P, NC], F32)
        bu_sb = cpool.tile([P, NC], F32)
        bs_sb = cpool.tile([P, NC], F32)
        v_sb = cpool.tile([P, NC], FP16)
        mask_sb = cpool.tile([P, NC, TK], F32)

        qeng = [nc.sync, nc.gpsimd, nc.scalar, nc.sync]
        # critical path first: kp needs text3/wkv/battn; qp needs audio/wq
        nc.scalar.dma_start(text3_sb[:], text3[:])
        nc.scalar.dma_start(battn_sb[:], battn_c[:])
        nc.scalar.dma_start(v_sb[:], v_c[:])
        for c in range(NC):
            qeng[c].dma_start(wkv_sb[:, c, :], wkv3[:, c, :])
        for c in range(NC):
            qeng[c].dma_start(audio_sb[:, c, :], audio3[:, c, :])
        for c in range(NC):
            qeng[(c + 2) % NC].dma_start(wq_sb[:, c, :], wq3[:, c, :])
        nc.scalar.dma_start(bu_sb[:], bu_c[:])
        for c in range(NC):
            qeng[c].dma_start(wu_sb[:, c, :], wu3[:, c, :])
        nc.scalar.dma_start(mask_sb[:], mask3[:])
        nc.scalar.dma_start(bs_sb[:], bs_c[:])
        nc.scalar.dma_start(text_sb[:], text2[:])
        for c in range(NC):
            qeng[(c + 2) % NC].dma_start(ws_sb[:, c, :], ws3[:, c, :])

        ident = cpool.tile([P, P], F32)
        masks.make_identity(nc, ident[:])

        # ---- projections ----
        kp_sb = cpool.tile([P, NC, TK], F32)
        for dc in range(NC):
            kp_ps = ppool.tile([P, TK], F32, tag="ps")
            for ec in range(NC):
                nc.tensor.matmul(
                    kp_ps[:],
                    wkv_sb[:, ec, dc * P:(dc + 1) * P],
                    text3_sb[:, ec, :],
                    start=(ec == 0),
                    stop=(ec == NC - 1),
                )
            nc.scalar.activation(
                kp_sb[:, dc, :], kp_ps[:], AF.Identity, bias=battn_sb[:, dc:dc + 1]
            )

        qp_t = []
        for dc in range(NC):
            qp_ps = ppool.tile([P, TQ], F32, tag="ps")
            for ec in range(NC):
                nc.tensor.matmul(
                    qp_ps[:],
                    wq_sb[:, ec, dc * P:(dc + 1) * P],
                    audio_sb[:, ec, :],
                    start=(ec == 0),
                    stop=(ec == NC - 1),
                )
            q = cpool.tile([P, TQ], FP16, tag=f"qp{dc}")
            nc.vector.tensor_copy(q[:], qp_ps[:])
            qp_t.append(q)

        # ---- g_u early: only needs audio + wu; runs while tanh loop owns ACT later ----
        gu_sb = cpool.tile([P, NC, TQ], F32)
        for dc in range(NC):
            gu_ps = ppool.tile([P, TQ], F32, tag="ps")
            for ec in range(NC):
                nc.tensor.matmul(
                    gu_ps[:],
                    wu_sb[:, ec, dc * P:(dc + 1) * P],
                    audio_sb[:, ec, :],
                    start=(ec == 0),
                    stop=(ec == NC - 1),
                )
            nc.scalar.activation(
                gu_sb[:, dc, :], gu_ps[:], AF.Sigmoid, bias=bu_sb[:, dc:dc + 1]
            )

        # ---- scores: score[q, k] = v . tanh(qp_T[:,q] + kp_T[:,k]) ----
        # lhsT = H chunk [128d, 128q] in bf16 (1 cyc/col weight load), rhs =
        # v chunk [128,1] bf16; accumulates [128q, 1] per (k, qc) over the 4
        # d-chunks directly into the [q, k]-layout PSUM score bank.
        score_ps = spool.tile([P, NC, TK], F32)
        for kb in range(TK // KB):
            x_t = xpool.tile([P, KB, NC, TQ], FP16, tag="x")
            for kk in range(KB):
                k = kb * KB + kk
                for dc in range(NC):
                    nc.vector.tensor_scalar_add(
                        x_t[:, kk, dc, :], qp_t[dc][:], kp_sb[:, dc, k:k + 1]
                    )
            h_t = hpool.tile([P, KB, NC, TQ], FP16, tag="h")
            nc.scalar.activation(h_t[:], x_t[:], AF.Tanh)
            for kk in range(KB):
                k = kb * KB + kk
                for qc in range(NC):
                    for dc in range(NC):
                        nc.tensor.matmul(
                            score_ps[:, qc, k:k + 1],
                            h_t[:, kk, dc, qc * P:(qc + 1) * P],
                            v_sb[:, dc:dc + 1],
                            start=(dc == 0),
                            stop=(dc == NC - 1),
                        )

        # ---- softmax over k (already in [q, k] layout) ----
        sm_sb = cpool.tile([P, NC, TK], F32)
        e_sb = cpool.tile([P, NC, TK], F32)
        attn_sb = cpool.tile([P, NC, TK], F32)
        attnT_sb = cpool.tile([TK, TQ], F32R)
        for qc in range(NC):
            nc.vector.tensor_add(sm_sb[:, qc, :], score_ps[:, qc, :], mask_sb[:, qc, :])
            nmax = wpool.tile([P, 1], F32, tag="nmax")
            nc.vector.reduce_max(
                nmax[:], sm_sb[:, qc, :], axis=mybir.AxisListType.X, negate=True
            )
            nc.scalar.activation(e_sb[:, qc, :], sm_sb[:, qc, :], AF.Exp, bias=nmax[:])
            ssum = wpool.tile([P, 1], F32, tag="ssum")
            nc.vector.reduce_sum(ssum[:], e_sb[:, qc, :], axis=mybir.AxisListType.X)
            rinv = wpool.tile([P, 1], F32, tag="rinv")
            nc.vector.reciprocal(rinv[:], ssum[:])
            nc.vector.tensor_scalar_mul(attn_sb[:, qc, :], e_sb[:, qc, :], rinv[:])
            at_ps = ppool.tile([TK, P], F32, tag="ps")
            nc.tensor.transpose(at_ps[:], attn_sb[:, qc, :], ident[:])
            nc.vector.tensor_copy(attnT_sb[:, qc * P:(qc + 1) * P], at_ps[:])

        # ---- ctx_T[e, q] = text.T @ attn_T ----
        ctx_sb = cpool.tile([P, NC, TQ], F32R)
        for ec in range(NC):
            ctx_ps = ppool.tile([P, TQ], F32, tag="ps")
            nc.tensor.matmul(
                ctx_ps[:],
                text_sb[:, ec * P:(ec + 1) * P],
                attnT_sb[:],
                start=True,
                stop=True,
            )
            nc.scalar.activation(ctx_sb[:, ec, :], ctx_ps[:], AF.Identity)

        # ---- gating tail: s_out = ctx*g_u (g_u precomputed); g_s from ctx ----
        for dc in range(NC):
            so_sb = wpool.tile([P, TQ], F32, tag="so")
            nc.vector.tensor_mul(so_sb[:], ctx_sb[:, dc, :].bitcast(F32), gu_sb[:, dc, :])
            (nc.sync if dc % 2 == 0 else nc.gpsimd).dma_start(soutT[:, dc, :], so_sb[:])

        for dc in range(NC):
            gs_ps = ppool.tile([P, TQ], F32, tag="ps")
            for ec in range(NC):
                nc.tensor.matmul(
                    gs_ps[:],
                    ws_sb[:, ec, dc * P:(dc + 1) * P],
                    ctx_sb[:, ec, :],
                    start=(ec == 0),
                    stop=(ec == NC - 1),
                )
            gs_sb = wpool.tile([P, TQ], F32, tag="gs")
            nc.scalar.activation(gs_sb[:], gs_ps[:], AF.Sigmoid, bias=bs_sb[:, dc:dc + 1])
            uo_sb = wpool.tile([P, TQ], F32, tag="uo")
            nc.vector.tensor_mul(uo_sb[:], audio_sb[:, dc, :].bitcast(F32), gs_sb[:])
            (nc.sync if dc % 2 == 0 else nc.gpsimd).dma_start(uoutT[:, dc, :], uo_sb[:])

    nc.compile()
    return nc


def _chunk_pd(x):
    """[D, F] -> [P, NC, F] with [p, c, f] = x[c*P + p, f]."""
    f = x.shape[1]
    return np.ascontiguousarray(
        x.reshape(NC, P, f).transpose(1, 0, 2), dtype=np.float32
    )


def _chunk_vec(x):
    """[D] -> [P, NC] with [p, c] = x[c*P + p]."""
    return np.ascontiguousarray(x.reshape(NC, P).T, dtype=np.float32)


def kernel(audio_emb, text_emb, audio_len, text_len,
           W_attn, b_attn, v, W_u, b_u, W_s, b_s):
    global _cached_nc, LAST_EXEC_NS
    audio_emb = np.asarray(audio_emb, dtype=np.float32)
    text_emb = np.asarray(text_emb, dtype=np.float32)
    audio_len = np.asarray(audio_len)
    text_len = np.asarray(text_len)
    W_attn = np.asarray(W_attn, dtype=np.float32)
    b_attn = np.asarray(b_attn, dtype=np.float32)
    v = np.asarray(v, dtype=np.float32)
    W_u = np.asarray(W_u, dtype=np.float32)
    b_u = np.asarray(b_u, dtype=np.float32)
    W_s = np.asarray(W_s, dtype=np.float32)
    b_s = np.asarray(b_s, dtype=np.float32)

    wq3 = _chunk_pd(W_attn[:, :D].T)
    wkv3 = _chunk_pd(W_attn[:, D:].T)
    wu3 = _chunk_pd(W_u.T)
    ws3 = _chunk_pd(W_s.T)
    battn_c = _chunk_vec(b_attn)
    bu_c = _chunk_vec(b_u)
    bs_c = _chunk_vec(b_s)
    v_c = _chunk_vec(v).astype(np.float16)

    q_ar = np.arange(TQ)
    k_ar = np.arange(TK)
    in_maps = []
    for b in range(B):
        valid = (q_ar[:, None] < int(audio_len[b])) & (k_ar[None, :] < int(text_len[b]))
        mask = np.where(valid, np.float32(0.0), np.float32(NEG)).astype(np.float32)
        in_maps.append({
            "audio3": _chunk_pd(audio_emb[b].T),
            "wq3": wq3,
            "wkv3": wkv3,
            "wu3": wu3,
            "ws3": ws3,
            "text2": np.ascontiguousarray(text_emb[b], dtype=np.float32),
            "text3": np.ascontiguousarray(
                text_emb[b].T.reshape(NC, P, TK).transpose(1, 0, 2), dtype=np.float32
            ),
            "battn_c": battn_c,
            "bu_c": bu_c,
            "bs_c": bs_c,
            "v_c": v_c,
            "mask3": np.ascontiguousarray(
                mask.reshape(NC, P, TK).transpose(1, 0, 2), dtype=np.float32
            ),
        })

    if _cached_nc is None:
        _cached_nc = _build()
    res = run_bass_kernel_spmd(_cached_nc, in_maps, list(range(B)), trace=TRACE)
    LAST_EXEC_NS = res.exec_time_ns

    u_out = np.empty((B, TQ, D), dtype=np.float32)
    s_out = np.empty((B, TQ, D), dtype=np.float32)
    for b in range(B):
        uT = res.results[b]["uoutT"].transpose(1, 0, 2).reshape(D, TQ)
        sT = res.results[b]["soutT"].transpose(1, 0, 2).reshape(D, TQ)
        u_out[b] = uT.T
        s_out[b] = sT.T
    return (u_out, s_out)


# revision 12
# speedup vs baseline: 1.7283x; 1.0600x over previous
"""Trainium2 Bass kernel for nn_CrossAttentionGating.

Sharding: data-parallel over batch B=8 across 8 cores (1 batch element per
core); all weights replicated. Host numpy does all layout prep (transposes,
chunking into 128-partition tiles, additive attention mask from lengths).

Per-core device pipeline (D=512 split into 4 chunks of 128 partitions):
  qp_T[d,q] = Wq^T.T @ audio^T          (PE, f32r)
  kp_T[d,k] = Wkv^T.T @ text^T + b_attn (PE + ACT bias)
  for each k:  X = qp_T + kp_T[:,k]     (DVE tensor_scalar per-partition add)
               H = tanh(X)              (ACT, batched big instructions)
               score_T[k,:] += v_c.T@H  (PE, M=1 f32r matmuls, PSUM accum)
  softmax over k: PE-transpose score to [q,k], +mask, max/exp/sum/recip
  ctx_T[e,q] = text.T @ attn_T          (PE)
  g_u = sigmoid(Wu^T.T @ audio^T + b_u); s_out_T = ctx_T * g_u
  g_s = sigmoid(Ws^T.T @ ctx_T + b_s);   u_out_T = audio_T * g_s
"""

import sys

for _p in ("/opt/trn_rl_repo", "/opt/pypackages"):
    if _p not in sys.path:
        sys.path.append(_p)

from contextlib import ExitStack

import ml_dtypes
import numpy as np

import concourse.bacc as bacc
import concourse.tile as tile
import concourse.mybir as mybir
from concourse import masks
from concourse.bass_utils import run_bass_kernel_spmd

B, TQ, TK, D = 8, 512, 64, 512
P = 128
NC = D // P  # 4 chunks of the embedding dim
KB = 4      # k's per tanh batch
NEG = -1e10
F32 = mybir.dt.float32
F32R = mybir.dt.float32r
BF16 = mybir.dt.bfloat16
FP16 = mybir.dt.float16
AF = mybir.ActivationFunctionType

TRACE = False
LAST_EXEC_NS = None

_cached_nc = None


def _build():
    nc = bacc.Bacc("TRN2", target_bir_lowering=False, debug=False, num_devices=B)

    audio3 = nc.dram_tensor("audio3", [P, NC, TQ], FP16, kind="ExternalInput")
    wq3 = nc.dram_tensor("wq3", [P, NC, D], FP16, kind="ExternalInput")
    wkv3 = nc.dram_tensor("wkv3", [P, NC, D], FP16, kind="ExternalInput")
    wu3 = nc.dram_tensor("wu3", [P, NC, D], FP16, kind="ExternalInput")
    ws3 = nc.dram_tensor("ws3", [P, NC, D], FP16, kind="ExternalInput")
    text2 = nc.dram_tensor("text2", [TK, D], FP16, kind="ExternalInput")
    text3 = nc.dram_tensor("text3", [P, NC, TK], FP16, kind="ExternalInput")
    battn_c = nc.dram_tensor("battn_c", [P, NC], F32, kind="ExternalInput")
    bu_c = nc.dram_tensor("bu_c", [P, NC], F32, kind="ExternalInput")
    bs_c = nc.dram_tensor("bs_c", [P, NC], F32, kind="ExternalInput")
    v_c = nc.dram_tensor("v_c", [P, NC], FP16, kind="ExternalInput")
    mask3 = nc.dram_tensor("mask3", [P, NC, TK], F32, kind="ExternalInput")
    uoutT = nc.dram_tensor("uoutT", [P, NC, TQ], F32, kind="ExternalOutput")
    soutT = nc.dram_tensor("soutT", [P, NC, TQ], F32, kind="ExternalOutput")

    with tile.TileContext(nc) as tc, ExitStack() as ctx:
        cpool = ctx.enter_context(tc.tile_pool(name="const", bufs=1))
        ppool = ctx.enter_context(tc.tile_pool(name="ps", bufs=4, space="PSUM"))
        spool = ctx.enter_context(tc.tile_pool(name="score", bufs=1, space="PSUM"))
        xpool = ctx.enter_context(tc.tile_pool(name="x", bufs=3))
        hpool = ctx.enter_context(tc.tile_pool(name="h", bufs=3))
        wpool = ctx.enter_context(tc.tile_pool(name="work", bufs=2))

        # ---- persistent loads (spread across per-engine DMA queues) ----
        audio_sb = cpool.tile([P, NC, TQ], FP16)
        wq_sb = cpool.tile([P, NC, D], FP16)
        wkv_sb = cpool.tile([P, NC, D], FP16)
        wu_sb = cpool.tile([P, NC, D], FP16)
        ws_sb = cpool.tile([P, NC, D], FP16)
        text_sb = cpool.tile([TK, D], FP16)
        text3_sb = cpool.tile([P, NC, TK], FP16)
        battn_sb = cpool.tile([P, NC], F32)
        bu_sb = cpool.tile([P, NC], F32)
        bs_sb = cpool.tile([P, NC], F32)
        v_sb = cpool.tile([P, NC], FP16)
        mask_sb = cpool.tile([P, NC, TK], F32)

        qeng = [nc.sync, nc.gpsimd, nc.scalar, nc.sync]
        # critical path first: kp needs text3/wkv/battn; qp needs audio/wq
        nc.scalar.dma_start(text3_sb[:], text3[:])
        nc.scalar.dma_start(battn_sb[:], battn_c[:])
        nc.scalar.dma_start(v_sb[:], v_c[:])
        for c in range(NC):
            qeng[c].dma_start(wkv_sb[:, c, :], wkv3[:, c, :])
        for c in range(NC):
            qeng[c].dma_start(audio_sb[:, c, :], audio3[:, c, :])
        for c in range(NC):
            qeng[(c + 2) % NC].dma_start(wq_sb[:, c, :], wq3[:, c, :])
        nc.scalar.dma_start(bu_sb[:], bu_c[:])
        for c in range(NC):
            qeng[c].dma_start(wu_sb[:, c, :], wu3[:, c, :])
        nc.scalar.dma_start(mask_sb[:], mask3[:])
        nc.scalar.dma_start(bs_sb[:], bs_c[:])
        nc.scalar.dma_start(text_sb[:], text2[:])
        for c in range(NC):
            qeng[(c + 2) % NC].dma_start(ws_sb[:, c, :], ws3[:, c, :])

        warm_sb = cpool.tile([1, 2], F32)
        nc.scalar.activation(warm_sb[:], warm_sb[:], AF.Exp)

        ident = cpool.tile([P, P], F32)
        masks.make_identity(nc, ident[:])

        # ---- projections ----
        kp_sb = cpool.tile([P, NC, TK], F32)
        for dc in range(NC):
            kp_ps = ppool.tile([P, TK], F32, tag="ps")
            for ec in range(NC):
                nc.tensor.matmul(
                    kp_ps[:],
                    wkv_sb[:, ec, dc * P:(dc + 1) * P],
                    text3_sb[:, ec, :],
                    start=(ec == 0),
                    stop=(ec == NC - 1),
                )
            nc.vector.tensor_scalar_add(
                kp_sb[:, dc, :], kp_ps[:], battn_sb[:, dc:dc + 1]
            )

        qp_t = []
        for dc in range(NC):
            qp_ps = ppool.tile([P, TQ], F32, tag="ps")
            for ec in range(NC):
                nc.tensor.matmul(
                    qp_ps[:],
                    wq_sb[:, ec, dc * P:(dc + 1) * P],
                    audio_sb[:, ec, :],
                    start=(ec == 0),
                    stop=(ec == NC - 1),
                )
            q = cpool.tile([P, TQ], FP16, tag=f"qp{dc}")
            nc.vector.tensor_copy(q[:], qp_ps[:])
            qp_t.append(q)

        # ---- g_u early: only needs audio + wu; runs while tanh loop owns ACT later ----
        gu_sb = cpool.tile([P, NC, TQ], F32)
        for dc in range(NC):
            gu_ps = ppool.tile([P, TQ], F32, tag="ps")
            for ec in range(NC):
                nc.tensor.matmul(
                    gu_ps[:],
                    wu_sb[:, ec, dc * P:(dc + 1) * P],
                    audio_sb[:, ec, :],
                    start=(ec == 0),
                    stop=(ec == NC - 1),
                )
            nc.scalar.activation(
                gu_sb[:, dc, :], gu_ps[:], AF.Sigmoid, bias=bu_sb[:, dc:dc + 1]
            )

        # ---- scores: score[q, k] = v . tanh(qp_T[:,q] + kp_T[:,k]) ----
        # lhsT = H chunk [128d, 128q] in bf16 (1 cyc/col weight load), rhs =
        # v chunk [128,1] bf16; accumulates [128q, 1] per (k, qc) over the 4
        # d-chunks directly into the [q, k]-layout PSUM score bank.
        score_ps = spool.tile([P, NC, TK], F32)
        for kb in range(TK // KB):
            x_t = xpool.tile([P, KB, NC, TQ], FP16, tag="x")
            for kk in range(KB):
                k = kb * KB + kk
                for dc in range(NC):
                    nc.vector.tensor_scalar_add(
                        x_t[:, kk, dc, :], qp_t[dc][:], kp_sb[:, dc, k:k + 1]
                    )
            h_t = hpool.tile([P, KB, NC, TQ], FP16, tag="h")
            nc.scalar.activation(h_t[:], x_t[:], AF.Tanh)
            for kk in range(KB):
                k = kb * KB + kk
                for qc in range(NC):
                    for dc in range(NC):
                        nc.tensor.matmul(
                            score_ps[:, qc, k:k + 1],
                            h_t[:, kk, dc, qc * P:(qc + 1) * P],
                            v_sb[:, dc:dc + 1],
                            start=(dc == 0),
                            stop=(dc == NC - 1),
                        )

        # ---- softmax over k (already in [q, k] layout) ----
        sm_sb = cpool.tile([P, NC, TK], F32)
        e_sb = cpool.tile([P, NC, TK], F32)
        attn_sb = cpool.tile([P, NC, TK], F32)
        attnT_sb = cpool.tile([TK, TQ], FP16)
        for qc in range(NC):
            nc.vector.tensor_add(sm_sb[:, qc, :], score_ps[:, qc, :], mask_sb[:, qc, :])
            nmax = wpool.tile([P, 1], F32, tag="nmax")
            nc.vector.reduce_max(
                nmax[:], sm_sb[:, qc, :], axis=mybir.AxisListType.X, negate=True
            )
            nc.scalar.activation(e_sb[:, qc, :], sm_sb[:, qc, :], AF.Exp, bias=nmax[:])
            ssum = wpool.tile([P, 1], F32, tag="ssum")
            nc.vector.reduce_sum(ssum[:], e_sb[:, qc, :], axis=mybir.AxisListType.X)
            rinv = wpool.tile([P, 1], F32, tag="rinv")
            nc.vector.reciprocal(rinv[:], ssum[:])
            nc.vector.tensor_scalar_mul(attn_sb[:, qc, :], e_sb[:, qc, :], rinv[:])
            at_ps = ppool.tile([TK, P], F32, tag="ps")
            nc.tensor.transpose(at_ps[:], attn_sb[:, qc, :], ident[:])
            nc.vector.tensor_copy(attnT_sb[:, qc * P:(qc + 1) * P], at_ps[:])

        # ---- ctx_T[e, q] = text.T @ attn_T ----
        ctx_sb = cpool.tile([P, NC, TQ], FP16)
        for ec in range(NC):
            ctx_ps = ppool.tile([P, TQ], F32, tag="ps")
            nc.tensor.matmul(
                ctx_ps[:],
                text_sb[:, ec * P:(ec + 1) * P],
                attnT_sb[:],
                start=True,
                stop=True,
            )
            nc.vector.tensor_copy(ctx_sb[:, ec, :], ctx_ps[:])

        # ---- gating tail: s_out = ctx*g_u (g_u precomputed); g_s from ctx ----
        for dc in range(NC):
            so_sb = wpool.tile([P, TQ], F32, tag="so")
            nc.vector.tensor_mul(so_sb[:], ctx_sb[:, dc, :], gu_sb[:, dc, :])
            (nc.sync if dc % 2 == 0 else nc.gpsimd).dma_start(soutT[:, dc, :], so_sb[:])

        for dc in range(NC):
            gs_ps = ppool.tile([P, TQ], F32, tag="ps")
            for ec in range(NC):
                nc.tensor.matmul(
                    gs_ps[:],
                    ws_sb[:, ec, dc * P:(dc + 1) * P],
                    ctx_sb[:, ec, :],
                    start=(ec == 0),
                    stop=(ec == NC - 1),
                )
            gs_sb = wpool.tile([P, TQ], F32, tag="gs")
            nc.scalar.activation(gs_sb[:], gs_ps[:], AF.Sigmoid, bias=bs_sb[:, dc:dc + 1])
            uo_sb = wpool.tile([P, TQ], F32, tag="uo")
            nc.vector.tensor_mul(uo_sb[:], audio_sb[:, dc, :], gs_sb[:])
            (nc.sync if dc % 2 == 0 else nc.gpsimd).dma_start(uoutT[:, dc, :], uo_sb[:])

    nc.compile()
    return nc


def _chunk_pd(x, dt=np.float16):
    """[D, F] -> [P, NC, F] with [p, c, f] = x[c*P + p, f]."""
    f = x.shape[1]
    return np.ascontiguousarray(
        x.reshape(NC, P, f).transpose(1, 0, 2), dtype=dt
    )


def _chunk_vec(x):
    """[D] -> [P, NC] with [p, c] = x[c*P + p]."""
    return np.ascontiguousarray(x.reshape(NC, P).T, dtype=np.float32)


def kernel(audio_emb, text_emb, audio_len, text_len,
           W_attn, b_attn, v, W_u, b_u, W_s, b_s):
    global _cached_nc, LAST_EXEC_NS
    audio_emb = np.asarray(audio_emb, dtype=np.float32)
    text_emb = np.asarray(text_emb, dtype=np.float32)
    audio_len = np.asarray(audio_len)
    text_len = np.asarray(text_len)
    W_attn = np.asarray(W_attn, dtype=np.float32)
    b_attn = np.asarray(b_attn, dtype=np.float32)
    v = np.asarray(v, dtype=np.float32)
    W_u = np.asarray(W_u, dtype=np.float32)
    b_u = np.asarray(b_u, dtype=np.float32)
    W_s = np.asarray(W_s, dtype=np.float32)
    b_s = np.asarray(b_s, dtype=np.float32)

    wq3 = _chunk_pd(W_attn[:, :D].T)
    wkv3 = _chunk_pd(W_attn[:, D:].T)
    wu3 = _chunk_pd(W_u.T)
    ws3 = _chunk_pd(W_s.T)
    battn_c = _chunk_vec(b_attn)
    bu_c = _chunk_vec(b_u)
    bs_c = _chunk_vec(b_s)
    v_c = _chunk_vec(v).astype(np.float16)

    q_ar = np.arange(TQ)
    k_ar = np.arange(TK)
    in_maps = []
    for b in range(B):
        valid = (q_ar[:, None] < int(audio_len[b])) & (k_ar[None, :] < int(text_len[b]))
        mask = np.where(valid, np.float32(0.0), np.float32(NEG)).astype(np.float32)
        in_maps.append({
            "audio3": _chunk_pd(audio_emb[b].T),
            "wq3": wq3,
            "wkv3": wkv3,
            "wu3": wu3,
            "ws3": ws3,
            "text2": np.ascontiguousarray(text_emb[b], dtype=np.float16),
            "text3": np.ascontiguousarray(
                text_emb[b].T.reshape(NC, P, TK).transpose(1, 0, 2), dtype=np.float16
            ),
            "battn_c": battn_c,
            "bu_c": bu_c,
            "bs_c": bs_c,
            "v_c": v_c,
            "mask3": np.ascontiguousarray(
                mask.reshape(NC, P, TK).transpose(1, 0, 2), dtype=np.float32
            ),
        })

    if _cached_nc is None:
        _cached_nc = _build()
    res = run_bass_kernel_spmd(_cached_nc, in_maps, list(range(B)), trace=TRACE)
    LAST_EXEC_NS = res.exec_time_ns

    u_out = np.empty((B, TQ, D), dtype=np.float32)
    s_out = np.empty((B, TQ, D), dtype=np.float32)
    for b in range(B):
        uT = res.results[b]["uoutT"].transpose(1, 0, 2).reshape(D, TQ)
        sT = res.results[b]["soutT"].transpose(1, 0, 2).reshape(D, TQ)
        u_out[b] = uT.T
        s_out[b] = sT.T
    return (u_out, s_out)


# revision 13
# speedup vs baseline: 1.7613x; 1.0191x over previous
"""Trainium2 Bass kernel for nn_CrossAttentionGating.

Sharding: data-parallel over batch B=8 across 8 cores (1 batch element per
core); all weights replicated. Host numpy does all layout prep (transposes,
chunking into 128-partition tiles, additive attention mask from lengths).

Per-core device pipeline (D=512 split into 4 chunks of 128 partitions):
  qp_T[d,q] = Wq^T.T @ audio^T          (PE, f32r)
  kp_T[d,k] = Wkv^T.T @ text^T + b_attn (PE + ACT bias)
  for each k:  X = qp_T + kp_T[:,k]     (DVE tensor_scalar per-partition add)
               H = tanh(X)              (ACT, batched big instructions)
               score_T[k,:] += v_c.T@H  (PE, M=1 f32r matmuls, PSUM accum)
  softmax over k: PE-transpose score to [q,k], +mask, max/exp/sum/recip
  ctx_T[e,q] = text.T @ attn_T          (PE)
  g_u = sigmoid(Wu^T.T @ audio^T + b_u); s_out_T = ctx_T * g_u
  g_s = sigmoid(Ws^T.T @ ctx_T + b_s);   u_out_T = audio_T * g_s
"""

import sys

for _p in ("/opt/trn_rl_repo", "/opt/pypackages"):
    if _p not in sys.path:
        sys.path.append(_p)

from contextlib import ExitStack

import ml_dtypes
import numpy as np

import concourse.bacc as bacc
import concourse.tile as tile
import concourse.mybir as mybir
from concourse import masks
from concourse.bass_utils import run_bass_kernel_spmd

B, TQ, TK, D = 8, 512, 64, 512
P = 128
NC = D // P  # 4 chunks of the embedding dim
KB = 4      # k's per tanh batch
NEG = -1e10
F32 = mybir.dt.float32
F32R = mybir.dt.float32r
BF16 = mybir.dt.bfloat16
FP16 = mybir.dt.float16
AF = mybir.ActivationFunctionType

TRACE = False
LAST_EXEC_NS = None

_cached_nc = None


def _build():
    nc = bacc.Bacc("TRN2", target_bir_lowering=False, debug=False, num_devices=B)

    audio3 = nc.dram_tensor("audio3", [P, NC, TQ], FP16, kind="ExternalInput")
    wq3 = nc.dram_tensor("wq3", [P, NC, D], FP16, kind="ExternalInput")
    wkv3 = nc.dram_tensor("wkv3", [P, NC, D], FP16, kind="ExternalInput")
    wu3 = nc.dram_tensor("wu3", [P, NC, D], FP16, kind="ExternalInput")
    ws3 = nc.dram_tensor("ws3", [P, NC, D], FP16, kind="ExternalInput")
    text2 = nc.dram_tensor("text2", [TK, D], FP16, kind="ExternalInput")
    text3 = nc.dram_tensor("text3", [P, NC, TK], FP16, kind="ExternalInput")
    battn_c = nc.dram_tensor("battn_c", [P, NC], F32, kind="ExternalInput")
    bu_c = nc.dram_tensor("bu_c", [P, NC], F32, kind="ExternalInput")
    bs_c = nc.dram_tensor("bs_c", [P, NC], F32, kind="ExternalInput")
    v_c = nc.dram_tensor("v_c", [P, NC], FP16, kind="ExternalInput")
    mask3 = nc.dram_tensor("mask3", [P, NC, TK], F32, kind="ExternalInput")
    uoutT = nc.dram_tensor("uoutT", [P, NC, TQ], F32, kind="ExternalOutput")
    soutT = nc.dram_tensor("soutT", [P, NC, TQ], F32, kind="ExternalOutput")

    with tile.TileContext(nc) as tc, ExitStack() as ctx:
        cpool = ctx.enter_context(tc.tile_pool(name="const", bufs=1))
        ppool = ctx.enter_context(tc.tile_pool(name="ps", bufs=4, space="PSUM"))
        spool = ctx.enter_context(tc.tile_pool(name="score", bufs=1, space="PSUM"))
        xpool = ctx.enter_context(tc.tile_pool(name="x", bufs=3))
        hpool = ctx.enter_context(tc.tile_pool(name="h", bufs=3))
        wpool = ctx.enter_context(tc.tile_pool(name="work", bufs=4))

        # ---- persistent loads (spread across per-engine DMA queues) ----
        audio_sb = cpool.tile([P, NC, TQ], FP16)
        wq_sb = cpool.tile([P, NC, D], FP16)
        wkv_sb = cpool.tile([P, NC, D], FP16)
        wu_sb = cpool.tile([P, NC, D], FP16)
        ws_sb = cpool.tile([P, NC, D], FP16)
        text_sb = cpool.tile([TK, D], FP16)
        text3_sb = cpool.tile([P, NC, TK], FP16)
        battn_sb = cpool.tile([P, NC], F32)
        bu_sb = cpool.tile([P, NC], F32)
        bs_sb = cpool.tile([P, NC], F32)
        v_sb = cpool.tile([P, NC], FP16)
        mask_sb = cpool.tile([P, NC, TK], F32)

        qeng = [nc.sync, nc.gpsimd, nc.scalar]
        # critical path first, round-robin across the 3 DMA rings:
        # qp needs audio/wq; kp needs text3/wkv/battn
        nc.sync.dma_start(text3_sb[:], text3[:])
        nc.gpsimd.dma_start(battn_sb[:], battn_c[:])
        nc.scalar.dma_start(v_sb[:], v_c[:])
        qi = 0
        for c in range(NC):
            for t_sb, t_dr in ((audio_sb, audio3), (wq_sb, wq3), (wkv_sb, wkv3)):
                qeng[qi % 3].dma_start(t_sb[:, c, :], t_dr[:, c, :])
                qi += 1
        nc.gpsimd.dma_start(bu_sb[:], bu_c[:])
        for c in range(NC):
            qeng[c % 3].dma_start(wu_sb[:, c, :], wu3[:, c, :])
        nc.scalar.dma_start(mask_sb[:], mask3[:])
        nc.gpsimd.dma_start(bs_sb[:], bs_c[:])
        nc.sync.dma_start(text_sb[:], text2[:])
        for c in range(NC):
            qeng[(c + 1) % 3].dma_start(ws_sb[:, c, :], ws3[:, c, :])

        ident = cpool.tile([P, P], F32)
        masks.make_identity(nc, ident[:])

        # ---- projections ----
        kp_sb = cpool.tile([P, NC, TK], F32)
        for dc in range(NC):
            kp_ps = ppool.tile([P, TK], F32, tag="ps")
            for ec in range(NC):
                nc.tensor.matmul(
                    kp_ps[:],
                    wkv_sb[:, ec, dc * P:(dc + 1) * P],
                    text3_sb[:, ec, :],
                    start=(ec == 0),
                    stop=(ec == NC - 1),
                )
            nc.vector.tensor_scalar_add(
                kp_sb[:, dc, :], kp_ps[:], battn_sb[:, dc:dc + 1]
            )

        qp_t = []
        for dc in range(NC):
            qp_ps = ppool.tile([P, TQ], F32, tag="ps")
            for ec in range(NC):
                nc.tensor.matmul(
                    qp_ps[:],
                    wq_sb[:, ec, dc * P:(dc + 1) * P],
                    audio_sb[:, ec, :],
                    start=(ec == 0),
                    stop=(ec == NC - 1),
                )
            q = cpool.tile([P, TQ], FP16, tag=f"qp{dc}")
            nc.vector.tensor_copy(q[:], qp_ps[:])
            qp_t.append(q)

        # ---- g_u early: only needs audio + wu; runs while tanh loop owns ACT later ----
        gu_sb = cpool.tile([P, NC, TQ], F32)
        for dc in range(NC):
            gu_ps = ppool.tile([P, TQ], F32, tag="ps")
            for ec in range(NC):
                nc.tensor.matmul(
                    gu_ps[:],
                    wu_sb[:, ec, dc * P:(dc + 1) * P],
                    audio_sb[:, ec, :],
                    start=(ec == 0),
                    stop=(ec == NC - 1),
                )
            nc.scalar.activation(
                gu_sb[:, dc, :], gu_ps[:], AF.Sigmoid, bias=bu_sb[:, dc:dc + 1]
            )

        # ---- scores: score[q, k] = v . tanh(qp_T[:,q] + kp_T[:,k]) ----
        # lhsT = H chunk [128d, 128q] in bf16 (1 cyc/col weight load), rhs =
        # v chunk [128,1] bf16; accumulates [128q, 1] per (k, qc) over the 4
        # d-chunks directly into the [q, k]-layout PSUM score bank.
        score_ps = spool.tile([P, NC, TK], F32)
        for kb in range(TK // KB):
            x_t = xpool.tile([P, KB, NC, TQ], FP16, tag="x")
            for kk in range(KB):
                k = kb * KB + kk
                for dc in range(NC):
                    nc.vector.tensor_scalar_add(
                        x_t[:, kk, dc, :], qp_t[dc][:], kp_sb[:, dc, k:k + 1]
                    )
            h_t = hpool.tile([P, KB, NC, TQ], FP16, tag="h")
            nc.scalar.activation(h_t[:], x_t[:], AF.Tanh)
            for kk in range(KB):
                k = kb * KB + kk
                for qc in range(NC):
                    for dc in range(NC):
                        nc.tensor.matmul(
                            score_ps[:, qc, k:k + 1],
                            h_t[:, kk, dc, qc * P:(qc + 1) * P],
                            v_sb[:, dc:dc + 1],
                            start=(dc == 0),
                            stop=(dc == NC - 1),
                        )

        # ---- softmax over k (already in [q, k] layout) ----
        sm_sb = cpool.tile([P, NC, TK], F32)
        e_sb = cpool.tile([P, NC, TK], F32)
        attn_sb = cpool.tile([P, NC, TK], F32)
        attnT_sb = cpool.tile([TK, TQ], FP16)
        for qc in range(NC):
            nc.vector.tensor_add(sm_sb[:, qc, :], score_ps[:, qc, :], mask_sb[:, qc, :])
            nmax = wpool.tile([P, 1], F32, tag="nmax")
            nc.vector.reduce_max(
                nmax[:], sm_sb[:, qc, :], axis=mybir.AxisListType.X, negate=True
            )
            nc.scalar.activation(e_sb[:, qc, :], sm_sb[:, qc, :], AF.Exp, bias=nmax[:])
            ssum = wpool.tile([P, 1], F32, tag="ssum")
            nc.vector.reduce_sum(ssum[:], e_sb[:, qc, :], axis=mybir.AxisListType.X)
            rinv = wpool.tile([P, 1], F32, tag="rinv")
            nc.vector.reciprocal(rinv[:], ssum[:])
            nc.vector.tensor_scalar_mul(attn_sb[:, qc, :], e_sb[:, qc, :], rinv[:])
            at_ps = ppool.tile([TK, P], F32, tag="ps")
            nc.tensor.transpose(at_ps[:], attn_sb[:, qc, :], ident[:])
            nc.vector.tensor_copy(attnT_sb[:, qc * P:(qc + 1) * P], at_ps[:])

        # ---- ctx_T[e, q] = text.T @ attn_T ----
        ctx_sb = cpool.tile([P, NC, TQ], FP16)
        for ec in range(NC):
            ctx_ps = ppool.tile([P, TQ], F32, tag="ps")
            nc.tensor.matmul(
                ctx_ps[:],
                text_sb[:, ec * P:(ec + 1) * P],
                attnT_sb[:],
                start=True,
                stop=True,
            )
            nc.vector.tensor_copy(ctx_sb[:, ec, :], ctx_ps[:])

        # ---- gating tail: s_out = ctx*g_u (g_u precomputed); g_s from ctx ----
        for dc in range(NC):
            so_sb = wpool.tile([P, TQ], F32, tag="so")
            nc.vector.tensor_mul(so_sb[:], ctx_sb[:, dc, :], gu_sb[:, dc, :])
            (nc.sync if dc % 2 == 0 else nc.gpsimd).dma_start(soutT[:, dc, :], so_sb[:])

        for dc in range(NC):
            gs_ps = ppool.tile([P, TQ], F32, tag="ps")
            for ec in range(NC):
                nc.tensor.matmul(
                    gs_ps[:],
                    ws_sb[:, ec, dc * P:(dc + 1) * P],
                    ctx_sb[:, ec, :],
                    start=(ec == 0),
                    stop=(ec == NC - 1),
                )
            gs_sb = wpool.tile([P, TQ], F32, tag="gs")
            nc.scalar.activation(gs_sb[:], gs_ps[:], AF.Sigmoid, bias=bs_sb[:, dc:dc + 1])
            uo_sb = wpool.tile([P, TQ], F32, tag="uo")
            nc.vector.tensor_mul(uo_sb[:], audio_sb[:, dc, :], gs_sb[:])
            (nc.sync if dc % 2 == 0 else nc.gpsimd).dma_start(uoutT[:, dc, :], uo_sb[:])

    nc.compile()
    return nc


def _chunk_pd(x, dt=np.float16):
    """[D, F] -> [P, NC, F] with [p, c, f] = x[c*P + p, f]."""
    f = x.shape[1]
    return np.ascontiguousarray(
        x.reshape(NC, P, f).transpose(1, 0, 2), dtype=dt
    )


def _chunk_vec(x):
    """[D] -> [P, NC] with [p, c] = x[c*P + p]."""
    return np.ascontiguousarray(x.reshape(NC, P).T, dtype=np.float32)


def kernel(audio_emb, text_emb, audio_len, text_len,
           W_attn, b_attn, v, W_u, b_u, W_s, b_s):
    global _cached_nc, LAST_EXEC_NS
    audio_emb = np.asarray(audio_emb, dtype=np.float32)
    text_emb = np.asarray(text_emb, dtype=np.float32)
    audio_len = np.asarray(audio_len)
    text_len = np.asarray(text_len)
    W_attn = np.asarray(W_attn, dtype=np.float32)
    b_attn = np.asarray(b_attn, dtype=np.float32)
    v = np.asarray(v, dtype=np.float32)
    W_u = np.asarray(W_u, dtype=np.float32)
    b_u = np.asarray(b_u, dtype=np.float32)
    W_s = np.asarray(W_s, dtype=np.float32)
    b_s = np.asarray(b_s, dtype=np.float32)

    wq3 = _chunk_pd(W_attn[:, :D].T)
    wkv3 = _chunk_pd(W_attn[:, D:].T)
    wu3 = _chunk_pd(W_u.T)
    ws3 = _chunk_pd(W_s.T)
    battn_c = _chunk_vec(b_attn)
    bu_c = _chunk_vec(b_u)
    bs_c = _chunk_vec(b_s)
    v_c = _chunk_vec(v).astype(np.float16)

    q_ar = np.arange(TQ)
    k_ar = np.arange(TK)
    in_maps = []
    for b in range(B):
        valid = (q_ar[:, None] < int(audio_len[b])) & (k_ar[None, :] < int(text_len[b]))
        mask = np.where(valid, np.float32(0.0), np.float32(NEG)).astype(np.float32)
        in_maps.append({
            "audio3": _chunk_pd(audio_emb[b].T),
            "wq3": wq3,
            "wkv3": wkv3,
            "wu3": wu3,
            "ws3": ws3,
            "text2": np.ascontiguousarray(text_emb[b], dtype=np.float16),
            "text3": np.ascontiguousarray(
                text_emb[b].T.reshape(NC, P, TK).transpose(1, 0, 2), dtype=np.float16
            ),
            "battn_c": battn_c,
            "bu_c": bu_c,
            "bs_c": bs_c,
            "v_c": v_c,
            "mask3": np.ascontiguousarray(
                mask.reshape(NC, P, TK).transpose(1, 0, 2), dtype=np.float32
            ),
        })

    if _cached_nc is None:
        _cached_nc = _build()
    res = run_bass_kernel_spmd(_cached_nc, in_maps, list(range(B)), trace=TRACE)
    LAST_EXEC_NS = res.exec_time_ns

    u_out = np.empty((B, TQ, D), dtype=np.float32)
    s_out = np.empty((B, TQ, D), dtype=np.float32)
    for b in range(B):
        uT = res.results[b]["uoutT"].transpose(1, 0, 2).reshape(D, TQ)
        sT = res.results[b]["soutT"].transpose(1, 0, 2).reshape(D, TQ)
        u_out[b] = uT.T
        s_out[b] = sT.T
    return (u_out, s_out)
